# revision 20
# baseline (speedup 1.0000x reference)
"""Bass/Trainium2 kernel for nn_CapsuleLayer (dynamic routing capsule layer).

Reference computation:
    inputs: [B=32, J=2048, I=64], W: [K=32, J=2048, D=32, I=64]
    inputs_hat[b,k,j,d] = sum_i inputs[b,j,i] * W[k,j,d,i]
    3 routing iterations (softmax over K), output = squash(s_2)  [B, K, D]

Sharding: J (input capsules) split 8 ways -> J_loc = 256 per core.
Routing softmax (over K) is fully local; only the per-iteration
s[b,k,d] = sum_j c*hat partial sums need a 64KB AllReduce.

v2 design notes:
  - W streamed partition-major ([128, 131072] fp16, 16KB contiguous per
    partition per chunk) -> ~356 GB/s vs ~168 GB/s for per-pair tiles.
  - Pass A matmuls use 2-j block-diagonal x stations [128, 64]: full
    128-row contraction, out rows 0-63 / 64-127 via tile_position col.
  - s0 (uniform-c first routing sum) accumulated on the PE from SBUF hat
    via diagonal lhsT matmuls (frees DVE completely during pass A).
  - Routing iterations: fp16 tensor ops in DVE 2x mode; d-reduction by
    5-level in-place pairwise tree; softmax small ops per chunk;
    software-pipelined so ScalarE exp latency hides under next chunk.

Device layouts (per core):
  wt   : [128, 131072] fp16  = [(jp,i), (pair, d, k)]   partition-major
  xst  : [128, 8192]   fp16  = [(jp,i), (pair, jp2, b)] block-diag stations
  hat  : [128, 64, 1024] fp16 = [(jj,b), group, (d,k)]
  s    : [32, 1024] fp32 = [b, (d,k)]
"""

import os
import sys
import numpy as np

import concourse.bass as bass
import concourse.mybir as mybir
import concourse.tile as tile
from concourse import bacc
from concourse import bass_utils

AF = mybir.ActivationFunctionType
ALU = mybir.AluOpType
F16 = mybir.dt.float16
F32 = mybir.dt.float32

EPS = 1e-07
N_CORES = 8
B = 32          # batch
J = 2048        # input capsules (total)
I = 64          # input capsule dim
K = 32          # output capsules
D = 32          # output capsule dim
JL = J // N_CORES          # 256 local input capsules
NPAIR = JL // 2            # 128 station pairs
NGRP = JL // 4             # 64 groups of 4 j's
DK = D * K                 # 1024

GPC = 8                    # groups per routing chunk
NCHUNK = NGRP // GPC       # 8 chunks
WGPC = 2                   # groups per W dma chunk (4 pairs, 8KB/partition)
NWCHUNK = NGRP // WGPC     # 32 W chunks
XGPC = 16                  # groups per x dma chunk (32 pairs, 4KB/partition)
NXCHUNK = NGRP // XGPC     # 4 x chunks


def build_program():
    nc = bacc.Bacc("TRN2", target_bir_lowering=False, debug=False,
                   enable_asserts=False, num_devices=N_CORES)

    wt = nc.dram_tensor("wt", [128, NPAIR * DK], F16, kind="ExternalInput").ap()
    xst = nc.dram_tensor("xst", [128, NPAIR * I], F16, kind="ExternalInput").ap()
    diag = nc.dram_tensor("diag", [128, B], F16, kind="ExternalInput").ap()
    out_d = nc.dram_tensor("out", [B, DK], F32, kind="ExternalOutput").ap()
    dbg = None
    if bool(int(os.environ.get("CAPS_DEBUG", "0"))):
        dbg = {
            "hat": nc.dram_tensor("dbg_hat", [128, NGRP * DK], F16,
                                  kind="ExternalOutput").ap(),
            "s0": nc.dram_tensor("dbg_s0", [B, DK], F32,
                                 kind="ExternalOutput").ap(),
        }

    with tile.TileContext(nc) as tc:
        _emit(tc, wt, xst, diag, out_d, dbg)
    nc.compile()
    return nc


def _emit(tc, wt, xst, diag, out_d, dbg=None):
    nc = tc.nc
    with (
        tc.tile_pool(name="hat", bufs=1) as hat_pool,
        tc.tile_pool(name="big", bufs=1) as big_pool,     # prod
        tc.tile_pool(name="wld", bufs=3) as wld_pool,     # W chunks / ch halves
        tc.tile_pool(name="xld", bufs=2) as x_pool,
        tc.tile_pool(name="u16", bufs=1) as u_pool,
        tc.tile_pool(name="smx", bufs=2) as smx_pool,     # expu32 / z / zr / c16
        tc.tile_pool(name="small", bufs=1) as small_pool,
        tc.tile_pool(name="obc", bufs=1) as obc_pool,
        tc.tile_pool(name="const", bufs=1) as const_pool,
        tc.tile_pool(name="hatps", bufs=2, space="PSUM") as hat_psum,
        tc.tile_pool(name="accps", bufs=2, space="PSUM") as acc_psum,
        tc.tile_pool(name="dram", bufs=8, space="DRAM") as dram_pool,
    ):
        # ---- constants ----
        diag16 = const_pool.tile([128, B], F16, tag="diag")
        nc.sync.dma_start(diag16[:], diag)

        # warm the collective stream (cc barrier + setup) off the critical path
        warm_in = dram_pool.tile([B, K], F16, name="warm_in")
        warm_out = dram_pool.tile([B, K], F16, name="warm_out")
        nc.sync.dma_start(warm_in[:], diag16[0:B, 0:K])
        nc.gpsimd.collective_compute(
            "AllReduce", ALU.add,
            replica_groups=[list(range(N_CORES))],
            ins=[warm_in.opt()],
            outs=[warm_out.opt()],
        )
        # warm the activation table sets (Exp first, Sqrt last so the first
        # squash finds the sqrt set resident)
        wtile0 = small_pool.tile([B, K], F32, tag="warm")
        nc.scalar.activation(wtile0[:], diag16[0:B, 0:K], AF.Exp)
        nc.scalar.activation(wtile0[:], wtile0[:], AF.Sqrt)

        # persistent hat storage: [(jj,b), group, (d,k)] fp16
        hat_sb = hat_pool.tile([128, NGRP, DK], F16, tag="hat")
        # O accumulator (sum of squash outputs over past iterations)
        o_acc = small_pool.tile([B, DK], F32, tag="oacc")

        # ---- Pass A: hat = x @ W on PE; s0 accumulated on DVE (fp16) ----
        s0_ps = acc_psum.tile([B, DK], F32, tag="sacc", name="s0_ps")
        s0a = const_pool.tile([128, DK], F16, tag="s0a")

        def dma_w(cw):
            wtile = wld_pool.tile([128, 2 * WGPC * DK], F16, tag="wld",
                                  name=f"w{cw}")
            nc.sync.dma_start(
                wtile[:], wt[:, cw * 2 * WGPC * DK:(cw + 1) * 2 * WGPC * DK])
            return wtile

        def dma_x(cx):
            xt = x_pool.tile([128, 2 * XGPC * I], F16, tag="x", name=f"x{cx}")
            nc.sync.dma_start(
                xt[:], xst[:, cx * 2 * XGPC * I:(cx + 1) * 2 * XGPC * I])
            return xt

        w_tiles = {0: dma_w(0), 1: dma_w(1)}
        x_tiles = {0: dma_x(0)}
        for g in range(NGRP):
            cx, cw = g // XGPC, g // WGPC
            # prefetch chunks two ahead (3 bufs)
            if g % WGPC == 0 and cw + 2 < NWCHUNK:
                w_tiles[cw + 2] = dma_w(cw + 2)
            if g % XGPC == 0 and cx + 1 < NXCHUNK:
                x_tiles[cx + 1] = dma_x(cx + 1)

            ps = hat_psum.tile([128, DK], F32, tag="hatps", name=f"hat_ps{g}")
            xt = x_tiles[cx]
            wtile = w_tiles[cw]
            for q in (0, 1):            # station pair within group
                pair = 2 * g + q
                xoff = (pair - 2 * XGPC * cx) * I
                woff = (pair - 2 * WGPC * cw) * DK
                for h in (0, 1):
                    nc.tensor.matmul(
                        ps[q * 64:(q + 1) * 64, h * 512:(h + 1) * 512],
                        lhsT=xt[:, xoff:xoff + I],
                        rhs=wtile[:, woff + h * 512:woff + (h + 1) * 512],
                        start=True, stop=True,
                        tile_position=(0, q * 64),
                        skip_group_check=True,
                    )
            # PSUM -> SBUF fp16 (split across ScalarE / VectorE)
            nc.scalar.copy(hat_sb[:, g, 0:512], ps[:, 0:512])
            nc.vector.tensor_copy(hat_sb[:, g, 512:DK], ps[:, 512:DK])
            # s0 partial accumulation (fp16, DVE 2x mode)
            if g == 0:
                nc.vector.tensor_copy(s0a[:], hat_sb[:, 0, :])
            else:
                nc.vector.tensor_add(s0a[:], s0a[:], hat_sb[:, g, :])

        # jj-sum of s0a on the PE via the diagonal station
        for h in (0, 1):
            nc.tensor.matmul(
                s0_ps[:, h * 512:(h + 1) * 512],
                lhsT=diag16[:],
                rhs=s0a[:, h * 512:(h + 1) * 512],
                start=True, stop=True,
                skip_group_check=True,
            )

        if dbg is not None:
            nc.sync.dma_start(dbg["hat"],
                              hat_sb.rearrange("p g f -> p (g f)"))

        s_ps = s0_ps
        # ---- routing iterations ----
        for r in range(3):
            # s partial -> AllReduce -> s_full
            s16 = small_pool.tile([B, DK], F16, tag="s16", name=f"s16_{r}")
            nc.scalar.copy(s16[:], s_ps[:, :])
            ar_in = dram_pool.tile([B, DK], F16, name=f"ar_in{r}")
            ar_out = dram_pool.tile([B, DK], F16, name=f"ar_out{r}")
            nc.sync.dma_start(ar_in[:], s16[:])
            nc.gpsimd.collective_compute(
                "AllReduce", ALU.add,
                replica_groups=[list(range(N_CORES))],
                ins=[ar_in.opt()],
                outs=[ar_out.opt()],
            )
            nc.sync.dma_start(s16[:], ar_out[:])
            if dbg is not None and r == 0:
                s_dbg = small_pool.tile([B, DK], F32, tag="sdbg")
                nc.vector.tensor_copy(s_dbg[:], s16[:])
                nc.sync.dma_start(dbg["s0"], s_dbg[:])

            # squash: scale = s2/(1+s2)/sqrt(s2+eps), per (b,k); s2 = sum_d s^2
            # For r==0 the softmax is uniform: s0 actual = s_full / K;
            # fold 1/K into the squared-norm (scale=1/K) and output scale.
            alpha = (1.0 / K) if r == 0 else 1.0
            sq = small_pool.tile([B, DK], F32, tag="sq")
            nc.scalar.activation(sq[:], s16[:], AF.Square, scale=alpha)
            s2 = small_pool.tile([B, K], F32, tag="s2")
            nc.vector.reduce_sum(s2[:], sq.rearrange("p (d k) -> p k d", d=D),
                                 axis=mybir.AxisListType.X)
            t2 = small_pool.tile([B, K], F32, tag="t2")
            nc.vector.tensor_scalar_add(t2[:], s2[:], EPS)
            nc.scalar.activation(t2[:], t2[:], AF.Sqrt)
            den = small_pool.tile([B, K], F32, tag="den")
            nc.vector.scalar_tensor_tensor(den[:], s2[:], 1.0, t2[:],
                                           ALU.add, ALU.mult)
            nc.vector.reciprocal(den[:], den[:])
            scl = small_pool.tile([B, K], F32, tag="scl")
            nc.vector.tensor_mul(scl[:], s2[:], den[:])
            if r == 0:
                nc.vector.tensor_scalar_mul(scl[:], scl[:], alpha)
            o_r = small_pool.tile([B, DK], F32, tag="sfull", name=f"o_r{r}")
            nc.vector.tensor_tensor(
                o_r.rearrange("p (d k) -> p d k", d=D),
                s16.rearrange("p (d k) -> p d k", d=D),
                scl[:, None, :].to_broadcast([B, D, K]),
                ALU.mult,
            )

            if r == 2:
                nc.sync.dma_start(out_d, o_r[:])
                break

            # O_acc += o_r ; build O_bcast fp16 [128, (d,k)]
            if r == 0:
                nc.vector.tensor_copy(o_acc[:], o_r[:])
            else:
                nc.vector.tensor_add(o_acc[:], o_acc[:], o_r[:])
            o16 = small_pool.tile([B, DK], F16, tag="o16", name=f"o16_{r}")
            nc.vector.tensor_copy(o16[:], o_acc[:])
            o_bc = obc_pool.tile([128, DK], F16, tag="obc", name=f"obc_{r}")
            for jj in range(4):
                nc.sync.dma_start(o_bc[jj * 32:(jj + 1) * 32, :], o16[:])

            # next-iteration s accumulator
            s_ps = acc_psum.tile([B, DK], F32, tag="sacc", name=f"s{r + 1}_ps")

            # routing pass over hat chunks (software-pipelined by one chunk)
            u16 = u_pool.tile([128, NGRP, K], F16, tag="u16", name=f"u16_{r}")
            pend = None   # (ci, expu32) from previous chunk
            for ci in range(NCHUNK):
                gsl = slice(ci * GPC, (ci + 1) * GPC)
                hat_c = hat_sb[:, gsl, :]
                # u = sum_d hat * O_acc  (fp16 mul + in-place pairwise tree)
                prod = big_pool.tile([128, GPC, DK], F16, tag="big",
                                     name=f"prod_{r}_{ci}")
                nc.vector.tensor_tensor(
                    prod[:], hat_c,
                    o_bc[:, None, :].to_broadcast([128, GPC, DK]),
                    ALU.mult,
                )
                p4 = prod.rearrange("p g (d k) -> p g d k", d=D)
                nc.vector.tensor_add(p4[:, :, 0:16, :], p4[:, :, 0:16, :],
                                     p4[:, :, 16:32, :])
                nc.vector.tensor_add(p4[:, :, 0:8, :], p4[:, :, 0:8, :],
                                     p4[:, :, 8:16, :])
                nc.vector.tensor_add(p4[:, :, 0:4, :], p4[:, :, 0:4, :],
                                     p4[:, :, 4:8, :])
                nc.vector.tensor_add(p4[:, :, 0:2, :], p4[:, :, 0:2, :],
                                     p4[:, :, 2:4, :])
                nc.vector.tensor_add(u16[:, gsl, :], p4[:, :, 0, :],
                                     p4[:, :, 1, :])
                # exp on ScalarE (f32 out; no max-subtraction needed)
                expu = smx_pool.tile([128, GPC, K], F32, tag="expu",
                                     name=f"expu_{r}_{ci}")
                nc.scalar.activation(expu[:], u16[:, gsl, :], AF.Exp)

                # finish previous chunk (its exp has long completed)
                if pend is not None:
                    _emit_ch(nc, tc, hat_sb, diag16, s_ps, smx_pool, wld_pool,
                             pend[0], pend[1], r)
                pend = (ci, expu)
            _emit_ch(nc, tc, hat_sb, diag16, s_ps, smx_pool, wld_pool,
                     pend[0], pend[1], r, last=True)


def _emit_ch(nc, tc, hat_sb, diag16, s_ps, smx_pool, wld_pool, ci, expu, r,
             last=False):
    z = smx_pool.tile([128, GPC], F32, tag="z", name=f"z_{r}_{ci}")
    nc.vector.reduce_sum(z[:], expu[:], axis=mybir.AxisListType.X)
    nc.vector.reciprocal(z[:], z[:])
    c16 = smx_pool.tile([128, GPC, K], F16, tag="c16", name=f"c16_{r}_{ci}")
    nc.vector.tensor_tensor(
        c16[:], expu[:], z[:, :, None].to_broadcast([128, GPC, K]),
        ALU.mult,
    )
    HG = 2 if last else GPC // 2
    nsub = GPC // HG
    for hf in range(nsub):
        g0 = ci * GPC + hf * HG
        gsl = slice(g0, g0 + HG)
        ch = wld_pool.tile([128, HG, DK], F16, tag="wld",
                           name=f"ch_{r}_{ci}_{hf}")
        nc.vector.tensor_tensor(
            ch.rearrange("p g (d k) -> p g d k", d=D),
            hat_sb[:, gsl, :].rearrange("p g (d k) -> p g d k", d=D),
            c16[:, hf * HG:(hf + 1) * HG, None, :].to_broadcast(
                [128, HG, D, K]),
            ALU.mult,
        )
        for gg in range(HG):
            g = g0 + gg
            for h in (0, 1):
                nc.tensor.matmul(
                    s_ps[:, h * 512:(h + 1) * 512],
                    lhsT=diag16[:],
                    rhs=ch[:, gg, h * 512:(h + 1) * 512],
                    start=(g == 0),
                    stop=(last and hf == nsub - 1 and gg == HG - 1),
                    skip_group_check=True,
                )


def pack_inputs(inputs, W):
    """Host-side shard + layout pack. Returns in_maps (one dict per core)."""
    diag = np.zeros((128, B), np.float16)
    for p in range(128):
        diag[p, p % B] = 1.0

    in_maps = []
    for c in range(N_CORES):
        jsl = slice(c * JL, (c + 1) * JL)
        # W: [K, J, D, I] -> [JL, I, D, K] -> pairs [(jp,i)=128, (d,k)]
        wc = np.ascontiguousarray(
            W[:, jsl].transpose(1, 3, 2, 0), dtype=np.float16
        )  # [JL, I, D, K]
        wpair = wc.reshape(NPAIR, 2 * I, DK)       # [pair, (jp,i), (d,k)]
        wt = np.ascontiguousarray(
            wpair.transpose(1, 0, 2)).reshape(128, NPAIR * DK)

        # x block-diag stations: [pair, (jp,i)=128, (jp2,b)=64]
        xc = inputs[:, jsl, :]  # [B, JL, I]
        xt = np.ascontiguousarray(xc.transpose(1, 2, 0)).astype(np.float16)
        xs4 = np.zeros((NPAIR, 128, I), np.float16)
        xs4[:, 0:I, 0:B] = xt[0::2]
        xs4[:, I:128, B:2 * B] = xt[1::2]
        xst = np.ascontiguousarray(
            xs4.transpose(1, 0, 2)).reshape(128, NPAIR * I)
        in_maps.append({"wt": wt, "xst": xst, "diag": diag})
    return in_maps


_CACHED_NC = None


def _install_ntff_hook():
    """Provide antenv.axon_hooks.get_axon_ntff_profile_hook when the agent
    image lacks it, by driving the injected libaxon_pjrt.so directly."""
    import types
    import ctypes
    import contextlib
    try:
        from antenv.axon_hooks import get_axon_ntff_profile_hook  # noqa: F401
        return True
    except ImportError:
        pass
    so_path = "/opt/axon/libaxon_pjrt.so"
    if not os.path.exists(so_path):
        return False
    lib = ctypes.CDLL(so_path)
    if not hasattr(lib, "axon_start_nrt_profile"):
        return False
    lib.axon_start_nrt_profile.argtypes = [
        ctypes.POINTER(ctypes.c_int64), ctypes.c_size_t]
    lib.axon_start_nrt_profile.restype = ctypes.c_int64
    lib.axon_stop_nrt_profile.argtypes = [ctypes.c_char_p]
    lib.axon_stop_nrt_profile.restype = ctypes.c_int64

    @contextlib.contextmanager
    def _hook(output_dir, device_ids):
        import jax
        jax.devices()
        if device_ids:
            ids = (ctypes.c_int64 * len(device_ids))(*device_ids)
            rc = lib.axon_start_nrt_profile(ids, len(device_ids))
        else:
            rc = lib.axon_start_nrt_profile(None, 0)
        if rc != 0:
            raise RuntimeError(f"axon_start_nrt_profile rc={rc}")
        try:
            yield
        finally:
            n = lib.axon_stop_nrt_profile(str(output_dir).encode())
            if n < 0:
                raise RuntimeError(f"axon_stop_nrt_profile rc={n}")

    import antenv
    mod = types.ModuleType("antenv.axon_hooks")
    mod.get_axon_ntff_profile_hook = lambda: _hook
    mod.set_axon_ntff_profile_hook = lambda h: None
    sys.modules["antenv.axon_hooks"] = mod
    antenv.axon_hooks = mod
    return True


def kernel(inputs, W):
    global _CACHED_NC
    inputs = np.asarray(inputs)
    W = np.asarray(W)
    if _CACHED_NC is None:
        _CACHED_NC = build_program()
    nc = _CACHED_NC
    in_maps = pack_inputs(inputs, W)
    trace = bool(int(os.environ.get("CAPS_TRACE", "0")))
    if trace:
        trace = _install_ntff_hook()
    res = bass_utils.run_bass_kernel_spmd(
        nc, in_maps, core_ids=list(range(N_CORES)), trace=trace,
    )
    kernel.last_results = res
    if trace and res.exec_time_ns is not None:
        print(f"HW exec time: {res.exec_time_ns} ns", file=sys.stderr)
        kernel.last_exec_time_ns = res.exec_time_ns
    out = res.results[0]["out"]  # [B, DK] fp32, identical on all cores
    return np.ascontiguousarray(
        out.reshape(B, D, K).transpose(0, 2, 1)
    ).astype(np.float32)


kernel.last_exec_time_ns = None
kernel.last_results = None


# revision 21
# speedup vs baseline: 1.0006x; 1.0006x over previous
"""Bass/Trainium2 kernel for nn_CapsuleLayer (dynamic routing capsule layer).

Reference computation:
    inputs: [B=32, J=2048, I=64], W: [K=32, J=2048, D=32, I=64]
    inputs_hat[b,k,j,d] = sum_i inputs[b,j,i] * W[k,j,d,i]
    3 routing iterations (softmax over K), output = squash(s_2)  [B, K, D]

Sharding: J (input capsules) split 8 ways -> J_loc = 256 per core.
Routing softmax (over K) is fully local; only the per-iteration
s[b,k,d] = sum_j c*hat partial sums need a 64KB AllReduce.

v2 design notes:
  - W streamed partition-major ([128, 131072] fp16, 16KB contiguous per
    partition per chunk) -> ~356 GB/s vs ~168 GB/s for per-pair tiles.
  - Pass A matmuls use 2-j block-diagonal x stations [128, 64]: full
    128-row contraction, out rows 0-63 / 64-127 via tile_position col.
  - s0 (uniform-c first routing sum) accumulated on the PE from SBUF hat
    via diagonal lhsT matmuls (frees DVE completely during pass A).
  - Routing iterations: fp16 tensor ops in DVE 2x mode; d-reduction by
    5-level in-place pairwise tree; softmax small ops per chunk;
    software-pipelined so ScalarE exp latency hides under next chunk.

Device layouts (per core):
  wt   : [128, 131072] fp16  = [(jp,i), (pair, d, k)]   partition-major
  xst  : [128, 8192]   fp16  = [(jp,i), (pair, jp2, b)] block-diag stations
  hat  : [128, 64, 1024] fp16 = [(jj,b), group, (d,k)]
  s    : [32, 1024] fp32 = [b, (d,k)]
"""

import os
import sys
import numpy as np

import concourse.bass as bass
import concourse.mybir as mybir
import concourse.tile as tile
from concourse import bacc
from concourse import bass_utils

AF = mybir.ActivationFunctionType
ALU = mybir.AluOpType
F16 = mybir.dt.float16
F32 = mybir.dt.float32

EPS = 1e-07
N_CORES = 8
B = 32          # batch
J = 2048        # input capsules (total)
I = 64          # input capsule dim
K = 32          # output capsules
D = 32          # output capsule dim
JL = J // N_CORES          # 256 local input capsules
NPAIR = JL // 2            # 128 station pairs
NGRP = JL // 4             # 64 groups of 4 j's
DK = D * K                 # 1024

GPC = 8                    # groups per routing chunk
NCHUNK = NGRP // GPC       # 8 chunks
WGPC = 2                   # groups per W dma chunk (4 pairs, 8KB/partition)
NWCHUNK = NGRP // WGPC     # 32 W chunks
XGPC = 16                  # groups per x dma chunk (32 pairs, 4KB/partition)
NXCHUNK = NGRP // XGPC     # 4 x chunks


def build_program():
    nc = bacc.Bacc("TRN2", target_bir_lowering=False, debug=False,
                   enable_asserts=False, num_devices=N_CORES)

    wt = nc.dram_tensor("wt", [128, NPAIR * DK], F16, kind="ExternalInput").ap()
    xst = nc.dram_tensor("xst", [128, NPAIR * I], F16, kind="ExternalInput").ap()
    diag = nc.dram_tensor("diag", [128, B], F16, kind="ExternalInput").ap()
    diagt = nc.dram_tensor("diagt", [B, 128], F16, kind="ExternalInput").ap()
    out_d = nc.dram_tensor("out", [B, DK], F32, kind="ExternalOutput").ap()
    dbg = None
    if bool(int(os.environ.get("CAPS_DEBUG", "0"))):
        dbg = {
            "hat": nc.dram_tensor("dbg_hat", [128, NGRP * DK], F16,
                                  kind="ExternalOutput").ap(),
            "s0": nc.dram_tensor("dbg_s0", [B, DK], F32,
                                 kind="ExternalOutput").ap(),
        }

    with tile.TileContext(nc) as tc:
        _emit(tc, wt, xst, diag, diagt, out_d, dbg)
    nc.compile()
    return nc


def _emit(tc, wt, xst, diag, diagt, out_d, dbg=None):
    nc = tc.nc
    with (
        tc.tile_pool(name="hat", bufs=1) as hat_pool,
        tc.tile_pool(name="big", bufs=1) as big_pool,     # prod
        tc.tile_pool(name="wld", bufs=3) as wld_pool,     # W chunks / ch halves
        tc.tile_pool(name="xld", bufs=2) as x_pool,
        tc.tile_pool(name="u16", bufs=1) as u_pool,
        tc.tile_pool(name="smx", bufs=2) as smx_pool,     # expu32 / z / zr / c16
        tc.tile_pool(name="small", bufs=1) as small_pool,
        tc.tile_pool(name="obc", bufs=1) as obc_pool,
        tc.tile_pool(name="const", bufs=1) as const_pool,
        tc.tile_pool(name="hatps", bufs=3, space="PSUM") as hat_psum,
        tc.tile_pool(name="accps", bufs=1, space="PSUM") as acc_psum,
        tc.tile_pool(name="dram", bufs=8, space="DRAM") as dram_pool,
    ):
        # ---- constants ----
        diag16 = const_pool.tile([128, B], F16, tag="diag")
        nc.sync.dma_start(diag16[:], diag)
        diagt16 = const_pool.tile([B, 128], F16, tag="diagt")
        nc.sync.dma_start(diagt16[:], diagt)

        # warm the collective stream (cc barrier + setup) off the critical path
        warm_in = dram_pool.tile([B, K], F16, name="warm_in")
        warm_out = dram_pool.tile([B, K], F16, name="warm_out")
        nc.sync.dma_start(warm_in[:], diag16[0:B, 0:K])
        nc.gpsimd.collective_compute(
            "AllReduce", ALU.add,
            replica_groups=[list(range(N_CORES))],
            ins=[warm_in.opt()],
            outs=[warm_out.opt()],
        )
        # warm the activation table sets (Exp first, Sqrt last so the first
        # squash finds the sqrt set resident)
        wtile0 = small_pool.tile([B, K], F32, tag="warm")
        nc.scalar.activation(wtile0[:], diag16[0:B, 0:K], AF.Exp)
        nc.scalar.activation(wtile0[:], wtile0[:], AF.Sqrt)

        # persistent hat storage: [(jj,b), group, (d,k)] fp16
        hat_sb = hat_pool.tile([128, NGRP, DK], F16, tag="hat")
        # O accumulator (sum of squash outputs over past iterations)
        o_acc = small_pool.tile([B, DK], F32, tag="oacc")

        # ---- Pass A: hat = x @ W on PE; s0 accumulated on DVE (fp16) ----
        s0_ps = acc_psum.tile([B, DK], F32, tag="sacc", name="s0_ps")
        s0a = const_pool.tile([128, DK], F16, tag="s0a")

        def dma_w(cw):
            wtile = wld_pool.tile([128, 2 * WGPC * DK], F16, tag="wld",
                                  name=f"w{cw}")
            nc.sync.dma_start(
                wtile[:], wt[:, cw * 2 * WGPC * DK:(cw + 1) * 2 * WGPC * DK])
            return wtile

        def dma_x(cx):
            xt = x_pool.tile([128, 2 * XGPC * I], F16, tag="x", name=f"x{cx}")
            nc.sync.dma_start(
                xt[:], xst[:, cx * 2 * XGPC * I:(cx + 1) * 2 * XGPC * I])
            return xt

        w_tiles = {0: dma_w(0), 1: dma_w(1)}
        x_tiles = {0: dma_x(0)}
        for g in range(NGRP):
            cx, cw = g // XGPC, g // WGPC
            # prefetch chunks two ahead (3 bufs)
            if g % WGPC == 0 and cw + 2 < NWCHUNK:
                w_tiles[cw + 2] = dma_w(cw + 2)
            if g % XGPC == 0 and cx + 1 < NXCHUNK:
                x_tiles[cx + 1] = dma_x(cx + 1)

            ps = hat_psum.tile([128, DK], F32, tag="hatps", name=f"hat_ps{g}")
            xt = x_tiles[cx]
            wtile = w_tiles[cw]
            for q in (0, 1):            # station pair within group
                pair = 2 * g + q
                xoff = (pair - 2 * XGPC * cx) * I
                woff = (pair - 2 * WGPC * cw) * DK
                for h in (0, 1):
                    nc.tensor.matmul(
                        ps[q * 64:(q + 1) * 64, h * 512:(h + 1) * 512],
                        lhsT=xt[:, xoff:xoff + I],
                        rhs=wtile[:, woff + h * 512:woff + (h + 1) * 512],
                        start=True, stop=True,
                        tile_position=(0, q * 64),
                        skip_group_check=True,
                    )
            # PSUM -> SBUF fp16 (split across ScalarE / VectorE)
            nc.scalar.copy(hat_sb[:, g, 0:640], ps[:, 0:640])
            nc.vector.tensor_copy(hat_sb[:, g, 640:DK], ps[:, 640:DK])
            # s0 partial accumulation (fp16, DVE 2x mode)
            if g == 0:
                nc.vector.tensor_copy(s0a[:], hat_sb[:, 0, :])
            else:
                nc.vector.tensor_add(s0a[:], s0a[:], hat_sb[:, g, :])

        # jj-sum of s0a on the PE via the diagonal station
        for h in (0, 1):
            nc.tensor.matmul(
                s0_ps[:, h * 512:(h + 1) * 512],
                lhsT=diag16[:],
                rhs=s0a[:, h * 512:(h + 1) * 512],
                start=True, stop=True,
                skip_group_check=True,
            )

        if dbg is not None:
            nc.sync.dma_start(dbg["hat"],
                              hat_sb.rearrange("p g f -> p (g f)"))

        s_ps = s0_ps
        # ---- routing iterations ----
        for r in range(3):
            # s partial -> AllReduce -> s_full
            s16 = small_pool.tile([B, DK], F16, tag="s16", name=f"s16_{r}")
            nc.scalar.copy(s16[:], s_ps[:, :])
            ar_in = dram_pool.tile([B, DK], F16, name=f"ar_in{r}")
            ar_out = dram_pool.tile([B, DK], F16, name=f"ar_out{r}")
            nc.sync.dma_start(ar_in[:], s16[:])
            nc.gpsimd.collective_compute(
                "AllReduce", ALU.add,
                replica_groups=[list(range(N_CORES))],
                ins=[ar_in.opt()],
                outs=[ar_out.opt()],
            )
            nc.sync.dma_start(s16[:], ar_out[:])
            if dbg is not None and r == 0:
                s_dbg = small_pool.tile([B, DK], F32, tag="sdbg")
                nc.vector.tensor_copy(s_dbg[:], s16[:])
                nc.sync.dma_start(dbg["s0"], s_dbg[:])

            # squash: scale = s2/(1+s2)/sqrt(s2+eps), per (b,k); s2 = sum_d s^2
            # For r==0 the softmax is uniform: s0 actual = s_full / K;
            # fold 1/K into the squared-norm (scale=1/K) and output scale.
            alpha = (1.0 / K) if r == 0 else 1.0
            sq = small_pool.tile([B, DK], F32, tag="sq")
            nc.scalar.activation(sq[:], s16[:], AF.Square, scale=alpha)
            s2 = small_pool.tile([B, K], F32, tag="s2")
            nc.vector.reduce_sum(s2[:], sq.rearrange("p (d k) -> p k d", d=D),
                                 axis=mybir.AxisListType.X)
            t2 = small_pool.tile([B, K], F32, tag="t2")
            nc.vector.tensor_scalar_add(t2[:], s2[:], EPS)
            nc.scalar.activation(t2[:], t2[:], AF.Sqrt)
            den = small_pool.tile([B, K], F32, tag="den")
            nc.vector.scalar_tensor_tensor(den[:], s2[:], 1.0, t2[:],
                                           ALU.add, ALU.mult)
            nc.vector.reciprocal(den[:], den[:])
            scl = small_pool.tile([B, K], F32, tag="scl")
            nc.vector.tensor_mul(scl[:], s2[:], den[:])
            if r == 0:
                nc.vector.tensor_scalar_mul(scl[:], scl[:], alpha)
            o_r = small_pool.tile([B, DK], F32, tag="sfull", name=f"o_r{r}")
            nc.vector.tensor_tensor(
                o_r.rearrange("p (d k) -> p d k", d=D),
                s16.rearrange("p (d k) -> p d k", d=D),
                scl[:, None, :].to_broadcast([B, D, K]),
                ALU.mult,
            )

            if r == 2:
                nc.sync.dma_start(out_d, o_r[:])
                break

            # O_acc += o_r ; build O_bcast fp16 [128, (d,k)]
            if r == 0:
                nc.vector.tensor_copy(o_acc[:], o_r[:])
            else:
                nc.vector.tensor_add(o_acc[:], o_acc[:], o_r[:])
            o16 = small_pool.tile([B, DK], F16, tag="o16", name=f"o16_{r}")
            nc.vector.tensor_copy(o16[:], o_acc[:])
            o_bc = obc_pool.tile([128, DK], F16, tag="obc", name=f"obc_{r}")
            obc_ps = hat_psum.tile([128, DK], F32, tag="hatps",
                                   name=f"obc_ps{r}")
            for h in (0, 1):
                nc.tensor.matmul(
                    obc_ps[:, h * 512:(h + 1) * 512],
                    lhsT=diagt16[:],
                    rhs=o16[:, h * 512:(h + 1) * 512],
                    start=True, stop=True,
                    skip_group_check=True,
                )
            nc.scalar.copy(o_bc[:], obc_ps[:])

            # next-iteration s accumulator
            s_ps = acc_psum.tile([B, DK], F32, tag="sacc", name=f"s{r + 1}_ps")

            # routing pass over hat chunks (software-pipelined by one chunk)
            u16 = u_pool.tile([128, NGRP, K], F16, tag="u16", name=f"u16_{r}")
            pend = None   # (ci, expu32) from previous chunk
            for ci in range(NCHUNK):
                gsl = slice(ci * GPC, (ci + 1) * GPC)
                hat_c = hat_sb[:, gsl, :]
                # u = sum_d hat * O_acc  (fp16 mul + in-place pairwise tree)
                prod = big_pool.tile([128, GPC, DK], F16, tag="big",
                                     name=f"prod_{r}_{ci}")
                nc.vector.tensor_tensor(
                    prod[:], hat_c,
                    o_bc[:, None, :].to_broadcast([128, GPC, DK]),
                    ALU.mult,
                )
                p4 = prod.rearrange("p g (d k) -> p g d k", d=D)
                nc.vector.tensor_add(p4[:, :, 0:16, :], p4[:, :, 0:16, :],
                                     p4[:, :, 16:32, :])
                nc.vector.tensor_add(p4[:, :, 0:8, :], p4[:, :, 0:8, :],
                                     p4[:, :, 8:16, :])
                nc.vector.tensor_add(p4[:, :, 0:4, :], p4[:, :, 0:4, :],
                                     p4[:, :, 4:8, :])
                nc.vector.tensor_add(p4[:, :, 0:2, :], p4[:, :, 0:2, :],
                                     p4[:, :, 2:4, :])
                nc.vector.tensor_add(u16[:, gsl, :], p4[:, :, 0, :],
                                     p4[:, :, 1, :])
                # exp on ScalarE (f32 out; no max-subtraction needed)
                expu = smx_pool.tile([128, GPC, K], F32, tag="expu",
                                     name=f"expu_{r}_{ci}")
                nc.scalar.activation(expu[:], u16[:, gsl, :], AF.Exp)

                # finish previous chunk (its exp has long completed)
                if pend is not None:
                    _emit_ch(nc, tc, hat_sb, diag16, s_ps, smx_pool, wld_pool,
                             pend[0], pend[1], r)
                pend = (ci, expu)
            _emit_ch(nc, tc, hat_sb, diag16, s_ps, smx_pool, wld_pool,
                     pend[0], pend[1], r, last=True)


def _emit_ch(nc, tc, hat_sb, diag16, s_ps, smx_pool, wld_pool, ci, expu, r,
             last=False):
    z = smx_pool.tile([128, GPC], F32, tag="z", name=f"z_{r}_{ci}")
    nc.vector.reduce_sum(z[:], expu[:], axis=mybir.AxisListType.X)
    nc.vector.reciprocal(z[:], z[:])
    c16 = smx_pool.tile([128, GPC, K], F16, tag="c16", name=f"c16_{r}_{ci}")
    nc.vector.tensor_tensor(
        c16[:], expu[:], z[:, :, None].to_broadcast([128, GPC, K]),
        ALU.mult,
    )
    HG = 2 if last else GPC // 2
    nsub = GPC // HG
    for hf in range(nsub):
        g0 = ci * GPC + hf * HG
        gsl = slice(g0, g0 + HG)
        ch = wld_pool.tile([128, HG, DK], F16, tag="wld",
                           name=f"ch_{r}_{ci}_{hf}")
        nc.vector.tensor_tensor(
            ch.rearrange("p g (d k) -> p g d k", d=D),
            hat_sb[:, gsl, :].rearrange("p g (d k) -> p g d k", d=D),
            c16[:, hf * HG:(hf + 1) * HG, None, :].to_broadcast(
                [128, HG, D, K]),
            ALU.mult,
        )
        for gg in range(HG):
            g = g0 + gg
            for h in (0, 1):
                nc.tensor.matmul(
                    s_ps[:, h * 512:(h + 1) * 512],
                    lhsT=diag16[:],
                    rhs=ch[:, gg, h * 512:(h + 1) * 512],
                    start=(g == 0),
                    stop=(last and hf == nsub - 1 and gg == HG - 1),
                    skip_group_check=True,
                )


def pack_inputs(inputs, W):
    """Host-side shard + layout pack. Returns in_maps (one dict per core)."""
    diag = np.zeros((128, B), np.float16)
    for p in range(128):
        diag[p, p % B] = 1.0
    diagt = np.ascontiguousarray(diag.T)

    in_maps = []
    for c in range(N_CORES):
        jsl = slice(c * JL, (c + 1) * JL)
        # W: [K, J, D, I] -> [JL, I, D, K] -> pairs [(jp,i)=128, (d,k)]
        wc = np.ascontiguousarray(
            W[:, jsl].transpose(1, 3, 2, 0), dtype=np.float16
        )  # [JL, I, D, K]
        wpair = wc.reshape(NPAIR, 2 * I, DK)       # [pair, (jp,i), (d,k)]
        wt = np.ascontiguousarray(
            wpair.transpose(1, 0, 2)).reshape(128, NPAIR * DK)

        # x block-diag stations: [pair, (jp,i)=128, (jp2,b)=64]
        xc = inputs[:, jsl, :]  # [B, JL, I]
        xt = np.ascontiguousarray(xc.transpose(1, 2, 0)).astype(np.float16)
        xs4 = np.zeros((NPAIR, 128, I), np.float16)
        xs4[:, 0:I, 0:B] = xt[0::2]
        xs4[:, I:128, B:2 * B] = xt[1::2]
        xst = np.ascontiguousarray(
            xs4.transpose(1, 0, 2)).reshape(128, NPAIR * I)
        in_maps.append({"wt": wt, "xst": xst, "diag": diag, "diagt": diagt})
    return in_maps


_CACHED_NC = None


def _install_ntff_hook():
    """Provide antenv.axon_hooks.get_axon_ntff_profile_hook when the agent
    image lacks it, by driving the injected libaxon_pjrt.so directly."""
    import types
    import ctypes
    import contextlib
    try:
        from antenv.axon_hooks import get_axon_ntff_profile_hook  # noqa: F401
        return True
    except ImportError:
        pass
    so_path = "/opt/axon/libaxon_pjrt.so"
    if not os.path.exists(so_path):
        return False
    lib = ctypes.CDLL(so_path)
    if not hasattr(lib, "axon_start_nrt_profile"):
        return False
    lib.axon_start_nrt_profile.argtypes = [
        ctypes.POINTER(ctypes.c_int64), ctypes.c_size_t]
    lib.axon_start_nrt_profile.restype = ctypes.c_int64
    lib.axon_stop_nrt_profile.argtypes = [ctypes.c_char_p]
    lib.axon_stop_nrt_profile.restype = ctypes.c_int64

    @contextlib.contextmanager
    def _hook(output_dir, device_ids):
        import jax
        jax.devices()
        if device_ids:
            ids = (ctypes.c_int64 * len(device_ids))(*device_ids)
            rc = lib.axon_start_nrt_profile(ids, len(device_ids))
        else:
            rc = lib.axon_start_nrt_profile(None, 0)
        if rc != 0:
            raise RuntimeError(f"axon_start_nrt_profile rc={rc}")
        try:
            yield
        finally:
            n = lib.axon_stop_nrt_profile(str(output_dir).encode())
            if n < 0:
                raise RuntimeError(f"axon_stop_nrt_profile rc={n}")

    import antenv
    mod = types.ModuleType("antenv.axon_hooks")
    mod.get_axon_ntff_profile_hook = lambda: _hook
    mod.set_axon_ntff_profile_hook = lambda h: None
    sys.modules["antenv.axon_hooks"] = mod
    antenv.axon_hooks = mod
    return True


def kernel(inputs, W):
    global _CACHED_NC
    inputs = np.asarray(inputs)
    W = np.asarray(W)
    if _CACHED_NC is None:
        _CACHED_NC = build_program()
    nc = _CACHED_NC
    in_maps = pack_inputs(inputs, W)
    trace = bool(int(os.environ.get("CAPS_TRACE", "0")))
    if trace:
        trace = _install_ntff_hook()
    res = bass_utils.run_bass_kernel_spmd(
        nc, in_maps, core_ids=list(range(N_CORES)), trace=trace,
    )
    kernel.last_results = res
    if trace and res.exec_time_ns is not None:
        print(f"HW exec time: {res.exec_time_ns} ns", file=sys.stderr)
        kernel.last_exec_time_ns = res.exec_time_ns
    out = res.results[0]["out"]  # [B, DK] fp32, identical on all cores
    return np.ascontiguousarray(
        out.reshape(B, D, K).transpose(0, 2, 1)
    ).astype(np.float32)


kernel.last_exec_time_ns = None
kernel.last_results = None


# revision 22
# speedup vs baseline: 1.0126x; 1.0119x over previous
"""Bass/Trainium2 kernel for nn_CapsuleLayer (dynamic routing capsule layer).

Reference computation:
    inputs: [B=32, J=2048, I=64], W: [K=32, J=2048, D=32, I=64]
    inputs_hat[b,k,j,d] = sum_i inputs[b,j,i] * W[k,j,d,i]
    3 routing iterations (softmax over K), output = squash(s_2)  [B, K, D]

Sharding: J (input capsules) split 8 ways -> J_loc = 256 per core.
Routing softmax (over K) is fully local; only the per-iteration
s[b,k,d] = sum_j c*hat partial sums need a 64KB AllReduce.

v2 design notes:
  - W streamed partition-major ([128, 131072] fp16, 16KB contiguous per
    partition per chunk) -> ~356 GB/s vs ~168 GB/s for per-pair tiles.
  - Pass A matmuls use 2-j block-diagonal x stations [128, 64]: full
    128-row contraction, out rows 0-63 / 64-127 via tile_position col.
  - s0 (uniform-c first routing sum) accumulated on the PE from SBUF hat
    via diagonal lhsT matmuls (frees DVE completely during pass A).
  - Routing iterations: fp16 tensor ops in DVE 2x mode; d-reduction by
    5-level in-place pairwise tree; softmax small ops per chunk;
    software-pipelined so ScalarE exp latency hides under next chunk.

Device layouts (per core):
  wt   : [128, 131072] fp16  = [(jp,i), (pair, d, k)]   partition-major
  xst  : [128, 8192]   fp16  = [(jp,i), (pair, jp2, b)] block-diag stations
  hat  : [128, 64, 1024] fp16 = [(jj,b), group, (d,k)]
  s    : [32, 1024] fp32 = [b, (d,k)]
"""

import os
import sys
import numpy as np

import concourse.bass as bass
import concourse.mybir as mybir
import concourse.tile as tile
from concourse import bacc
from concourse import bass_utils

AF = mybir.ActivationFunctionType
ALU = mybir.AluOpType
F16 = mybir.dt.float16
F32 = mybir.dt.float32

EPS = 1e-07
N_CORES = 8
B = 32          # batch
J = 2048        # input capsules (total)
I = 64          # input capsule dim
K = 32          # output capsules
D = 32          # output capsule dim
JL = J // N_CORES          # 256 local input capsules
NPAIR = JL // 2            # 128 station pairs
NGRP = JL // 4             # 64 groups of 4 j's
DK = D * K                 # 1024

GPC = 8                    # groups per routing chunk
NCHUNK = NGRP // GPC       # 8 chunks
WGPC = 2                   # groups per W dma chunk (4 pairs, 8KB/partition)
NWCHUNK = NGRP // WGPC     # 32 W chunks
XGPC = 16                  # groups per x dma chunk (32 pairs, 4KB/partition)
NXCHUNK = NGRP // XGPC     # 4 x chunks


def build_program():
    nc = bacc.Bacc("TRN2", target_bir_lowering=False, debug=False,
                   enable_asserts=False, num_devices=N_CORES)

    wt = nc.dram_tensor("wt", [128, NPAIR * DK], F16, kind="ExternalInput").ap()
    xst = nc.dram_tensor("xst", [128, NPAIR * I], F16, kind="ExternalInput").ap()
    diag = nc.dram_tensor("diag", [128, B], F16, kind="ExternalInput").ap()
    diagt = nc.dram_tensor("diagt", [B, 128], F16, kind="ExternalInput").ap()
    out_d = nc.dram_tensor("out", [B, DK], F32, kind="ExternalOutput").ap()
    dbg = None
    if bool(int(os.environ.get("CAPS_DEBUG", "0"))):
        dbg = {
            "hat": nc.dram_tensor("dbg_hat", [128, NGRP * DK], F16,
                                  kind="ExternalOutput").ap(),
            "s0": nc.dram_tensor("dbg_s0", [B, DK], F32,
                                 kind="ExternalOutput").ap(),
        }

    with tile.TileContext(nc) as tc:
        _emit(tc, wt, xst, diag, diagt, out_d, dbg)
    nc.compile()
    return nc


def _emit(tc, wt, xst, diag, diagt, out_d, dbg=None):
    nc = tc.nc
    with (
        tc.tile_pool(name="hat", bufs=1) as hat_pool,
        tc.tile_pool(name="big", bufs=1) as big_pool,     # prod
        tc.tile_pool(name="wld", bufs=3) as wld_pool,     # W chunks / ch halves
        tc.tile_pool(name="xld", bufs=2) as x_pool,
        tc.tile_pool(name="u16", bufs=1) as u_pool,
        tc.tile_pool(name="smx", bufs=2) as smx_pool,     # expu32 / z / zr / c16
        tc.tile_pool(name="small", bufs=1) as small_pool,
        tc.tile_pool(name="obc", bufs=1) as obc_pool,
        tc.tile_pool(name="const", bufs=1) as const_pool,
        tc.tile_pool(name="hatps", bufs=3, space="PSUM") as hat_psum,
        tc.tile_pool(name="accps", bufs=1, space="PSUM") as acc_psum,
        tc.tile_pool(name="dram", bufs=8, space="DRAM") as dram_pool,
    ):
        # ---- constants ----
        diag16 = const_pool.tile([128, B], F16, tag="diag")
        nc.sync.dma_start(diag16[:], diag)
        diagt16 = const_pool.tile([B, 128], F16, tag="diagt")
        nc.sync.dma_start(diagt16[:], diagt)

        # persistent hat storage: [(jj,b), group, (d,k)] fp16
        hat_sb = hat_pool.tile([128, NGRP, DK], F16, tag="hat")
        # O accumulator (sum of squash outputs over past iterations)
        o_acc = small_pool.tile([B, DK], F16, tag="oacc")

        # ---- Pass A: hat = x @ W on PE; s0 accumulated on DVE (fp16) ----
        s0_ps = acc_psum.tile([B, DK], F32, tag="sacc", name="s0_ps")
        s0a = const_pool.tile([128, DK], F16, tag="s0a")

        def dma_w(cw):
            wtile = wld_pool.tile([128, 2 * WGPC * DK], F16, tag="wld",
                                  name=f"w{cw}")
            nc.sync.dma_start(
                wtile[:], wt[:, cw * 2 * WGPC * DK:(cw + 1) * 2 * WGPC * DK])
            return wtile

        def dma_x(cx):
            xt = x_pool.tile([128, 2 * XGPC * I], F16, tag="x", name=f"x{cx}")
            nc.sync.dma_start(
                xt[:], xst[:, cx * 2 * XGPC * I:(cx + 1) * 2 * XGPC * I])
            return xt

        w_tiles = {0: dma_w(0), 1: dma_w(1)}
        x_tiles = {0: dma_x(0)}

        # warm the collective stream (cc barrier + setup) off the critical path
        warm_in = dram_pool.tile([B, K], F16, name="warm_in")
        warm_out = dram_pool.tile([B, K], F16, name="warm_out")
        nc.sync.dma_start(warm_in[:], diag16[0:B, 0:K])
        nc.gpsimd.collective_compute(
            "AllReduce", ALU.add,
            replica_groups=[list(range(N_CORES))],
            ins=[warm_in.opt()],
            outs=[warm_out.opt()],
        )
        # warm the activation table sets (Exp first, Sqrt last so the first
        # squash finds the sqrt set resident)
        wtile0 = small_pool.tile([B, K], F32, tag="warm")
        nc.scalar.activation(wtile0[:], diag16[0:B, 0:K], AF.Exp)
        nc.scalar.activation(wtile0[:], wtile0[:], AF.Sqrt)
        for g in range(NGRP):
            cx, cw = g // XGPC, g // WGPC
            # prefetch chunks two ahead (3 bufs)
            if g % WGPC == 0 and cw + 2 < NWCHUNK:
                w_tiles[cw + 2] = dma_w(cw + 2)
            if g % XGPC == 0 and cx + 1 < NXCHUNK:
                x_tiles[cx + 1] = dma_x(cx + 1)

            ps = hat_psum.tile([128, DK], F32, tag="hatps", name=f"hat_ps{g}")
            xt = x_tiles[cx]
            wtile = w_tiles[cw]
            for q in (0, 1):            # station pair within group
                pair = 2 * g + q
                xoff = (pair - 2 * XGPC * cx) * I
                woff = (pair - 2 * WGPC * cw) * DK
                for h in (0, 1):
                    nc.tensor.matmul(
                        ps[q * 64:(q + 1) * 64, h * 512:(h + 1) * 512],
                        lhsT=xt[:, xoff:xoff + I],
                        rhs=wtile[:, woff + h * 512:woff + (h + 1) * 512],
                        start=True, stop=True,
                        tile_position=(0, q * 64),
                        skip_group_check=True,
                    )
            # PSUM -> SBUF fp16 (split across ScalarE / VectorE)
            nc.scalar.copy(hat_sb[:, g, 0:640], ps[:, 0:640])
            nc.vector.tensor_copy(hat_sb[:, g, 640:DK], ps[:, 640:DK])
            # s0 partial accumulation (fp16, DVE 2x mode)
            if g == 0:
                nc.vector.tensor_copy(s0a[:], hat_sb[:, 0, :])
            else:
                nc.vector.tensor_add(s0a[:], s0a[:], hat_sb[:, g, :])

        # jj-sum of s0a on the PE via the diagonal station
        for h in (0, 1):
            nc.tensor.matmul(
                s0_ps[:, h * 512:(h + 1) * 512],
                lhsT=diag16[:],
                rhs=s0a[:, h * 512:(h + 1) * 512],
                start=True, stop=True,
                skip_group_check=True,
            )

        if dbg is not None:
            nc.sync.dma_start(dbg["hat"],
                              hat_sb.rearrange("p g f -> p (g f)"))

        s_ps = s0_ps
        # ---- routing iterations ----
        for r in range(3):
            # s partial -> AllReduce -> s_full
            s16 = small_pool.tile([B, DK], F16, tag="s16", name=f"s16_{r}")
            nc.scalar.copy(s16[:], s_ps[:, :])
            ar_in = dram_pool.tile([B, DK], F16, name=f"ar_in{r}")
            ar_out = dram_pool.tile([B, DK], F16, name=f"ar_out{r}")
            nc.sync.dma_start(ar_in[:], s16[:])
            nc.scalar.activation(wtile0[:], wtile0[:], AF.Sqrt)
            nc.gpsimd.collective_compute(
                "AllReduce", ALU.add,
                replica_groups=[list(range(N_CORES))],
                ins=[ar_in.opt()],
                outs=[ar_out.opt()],
            )
            nc.sync.dma_start(s16[:], ar_out[:])
            if dbg is not None and r == 0:
                s_dbg = small_pool.tile([B, DK], F32, tag="sdbg")
                nc.vector.tensor_copy(s_dbg[:], s16[:])
                nc.sync.dma_start(dbg["s0"], s_dbg[:])

            # squash: scale = s2/(1+s2)/sqrt(s2+eps), per (b,k); s2 = sum_d s^2
            # For r==0 the softmax is uniform: s0 actual = s_full / K;
            # fold 1/K into the squared-norm (scale=1/K) and output scale.
            alpha = (1.0 / K) if r == 0 else 1.0
            sq = small_pool.tile([B, DK], F32, tag="sq")
            nc.scalar.activation(sq[:], s16[:], AF.Square, scale=alpha)
            s2 = small_pool.tile([B, K], F32, tag="s2")
            nc.vector.reduce_sum(s2[:], sq.rearrange("p (d k) -> p k d", d=D),
                                 axis=mybir.AxisListType.X)
            t2 = small_pool.tile([B, K], F32, tag="t2")
            nc.vector.tensor_scalar_add(t2[:], s2[:], EPS)
            nc.scalar.activation(t2[:], t2[:], AF.Sqrt)
            den = small_pool.tile([B, K], F32, tag="den")
            nc.vector.scalar_tensor_tensor(den[:], s2[:], 1.0, t2[:],
                                           ALU.add, ALU.mult)
            nc.vector.reciprocal(den[:], den[:])
            scl = small_pool.tile([B, K], F32, tag="scl")
            nc.vector.tensor_mul(scl[:], s2[:], den[:])
            if r == 0:
                nc.vector.tensor_scalar_mul(scl[:], scl[:], alpha)
            if r == 2:
                o_r = small_pool.tile([B, DK], F32, tag="sfull", name="o_r2")
                nc.vector.tensor_tensor(
                    o_r.rearrange("p (d k) -> p d k", d=D),
                    s16.rearrange("p (d k) -> p d k", d=D),
                    scl[:, None, :].to_broadcast([B, D, K]),
                    ALU.mult,
                )
                nc.sync.dma_start(out_d, o_r[:])
                break

            # o_r fp16; O_acc (fp16) += o_r ; broadcast via PE to [128, (d,k)]
            o_r = small_pool.tile([B, DK], F16, tag="o16", name=f"o_r{r}")
            nc.vector.tensor_tensor(
                o_r.rearrange("p (d k) -> p d k", d=D),
                s16.rearrange("p (d k) -> p d k", d=D),
                scl[:, None, :].to_broadcast([B, D, K]),
                ALU.mult,
            )
            if r == 0:
                nc.vector.tensor_copy(o_acc[:], o_r[:])
            else:
                nc.vector.tensor_add(o_acc[:], o_acc[:], o_r[:])
            o_bc = obc_pool.tile([128, DK], F16, tag="obc", name=f"obc_{r}")
            obc_ps = hat_psum.tile([128, DK], F32, tag="hatps",
                                   name=f"obc_ps{r}")
            for h in (0, 1):
                nc.tensor.matmul(
                    obc_ps[:, h * 512:(h + 1) * 512],
                    lhsT=diagt16[:],
                    rhs=o_acc[:, h * 512:(h + 1) * 512],
                    start=True, stop=True,
                    skip_group_check=True,
                )
            nc.scalar.copy(o_bc[:], obc_ps[:])

            # next-iteration s accumulator
            s_ps = acc_psum.tile([B, DK], F32, tag="sacc", name=f"s{r + 1}_ps")

            # routing pass over hat chunks (software-pipelined by one chunk)
            u16 = u_pool.tile([128, NGRP, K], F16, tag="u16", name=f"u16_{r}")
            pend = None   # (ci, expu32) from previous chunk
            for ci in range(NCHUNK):
                gsl = slice(ci * GPC, (ci + 1) * GPC)
                hat_c = hat_sb[:, gsl, :]
                # u = sum_d hat * O_acc  (fp16 mul + in-place pairwise tree)
                prod = big_pool.tile([128, GPC, DK], F16, tag="big",
                                     name=f"prod_{r}_{ci}")
                nc.vector.tensor_tensor(
                    prod[:], hat_c,
                    o_bc[:, None, :].to_broadcast([128, GPC, DK]),
                    ALU.mult,
                )
                p4 = prod.rearrange("p g (d k) -> p g d k", d=D)
                nc.vector.tensor_add(p4[:, :, 0:16, :], p4[:, :, 0:16, :],
                                     p4[:, :, 16:32, :])
                nc.vector.tensor_add(p4[:, :, 0:8, :], p4[:, :, 0:8, :],
                                     p4[:, :, 8:16, :])
                nc.vector.tensor_add(p4[:, :, 0:4, :], p4[:, :, 0:4, :],
                                     p4[:, :, 4:8, :])
                nc.vector.tensor_add(p4[:, :, 0:2, :], p4[:, :, 0:2, :],
                                     p4[:, :, 2:4, :])
                nc.vector.tensor_add(u16[:, gsl, :], p4[:, :, 0, :],
                                     p4[:, :, 1, :])
                # exp on ScalarE (f32 out; no max-subtraction needed)
                expu = smx_pool.tile([128, GPC, K], F32, tag="expu",
                                     name=f"expu_{r}_{ci}")
                nc.scalar.activation(expu[:], u16[:, gsl, :], AF.Exp)

                # finish previous chunk (its exp has long completed)
                if pend is not None:
                    _emit_ch(nc, tc, hat_sb, diag16, s_ps, smx_pool, wld_pool,
                             pend[0], pend[1], r)
                pend = (ci, expu)
            _emit_ch(nc, tc, hat_sb, diag16, s_ps, smx_pool, wld_pool,
                     pend[0], pend[1], r, last=True)


def _emit_ch(nc, tc, hat_sb, diag16, s_ps, smx_pool, wld_pool, ci, expu, r,
             last=False):
    z = smx_pool.tile([128, GPC], F32, tag="z", name=f"z_{r}_{ci}")
    nc.vector.reduce_sum(z[:], expu[:], axis=mybir.AxisListType.X)
    nc.vector.reciprocal(z[:], z[:])
    c16 = smx_pool.tile([128, GPC, K], F16, tag="c16", name=f"c16_{r}_{ci}")
    nc.vector.tensor_tensor(
        c16[:], expu[:], z[:, :, None].to_broadcast([128, GPC, K]),
        ALU.mult,
    )
    HG = 2 if last else GPC // 2
    nsub = GPC // HG
    for hf in range(nsub):
        g0 = ci * GPC + hf * HG
        gsl = slice(g0, g0 + HG)
        ch = wld_pool.tile([128, HG, DK], F16, tag="wld",
                           name=f"ch_{r}_{ci}_{hf}")
        nc.vector.tensor_tensor(
            ch.rearrange("p g (d k) -> p g d k", d=D),
            hat_sb[:, gsl, :].rearrange("p g (d k) -> p g d k", d=D),
            c16[:, hf * HG:(hf + 1) * HG, None, :].to_broadcast(
                [128, HG, D, K]),
            ALU.mult,
        )
        for gg in range(HG):
            g = g0 + gg
            for h in (0, 1):
                nc.tensor.matmul(
                    s_ps[:, h * 512:(h + 1) * 512],
                    lhsT=diag16[:],
                    rhs=ch[:, gg, h * 512:(h + 1) * 512],
                    start=(g == 0),
                    stop=(last and hf == nsub - 1 and gg == HG - 1),
                    skip_group_check=True,
                )


def pack_inputs(inputs, W):
    """Host-side shard + layout pack. Returns in_maps (one dict per core)."""
    diag = np.zeros((128, B), np.float16)
    for p in range(128):
        diag[p, p % B] = 1.0
    diagt = np.ascontiguousarray(diag.T)

    in_maps = []
    for c in range(N_CORES):
        jsl = slice(c * JL, (c + 1) * JL)
        # W: [K, J, D, I] -> [JL, I, D, K] -> pairs [(jp,i)=128, (d,k)]
        wc = np.ascontiguousarray(
            W[:, jsl].transpose(1, 3, 2, 0), dtype=np.float16
        )  # [JL, I, D, K]
        wpair = wc.reshape(NPAIR, 2 * I, DK)       # [pair, (jp,i), (d,k)]
        wt = np.ascontiguousarray(
            wpair.transpose(1, 0, 2)).reshape(128, NPAIR * DK)

        # x block-diag stations: [pair, (jp,i)=128, (jp2,b)=64]
        xc = inputs[:, jsl, :]  # [B, JL, I]
        xt = np.ascontiguousarray(xc.transpose(1, 2, 0)).astype(np.float16)
        xs4 = np.zeros((NPAIR, 128, I), np.float16)
        xs4[:, 0:I, 0:B] = xt[0::2]
        xs4[:, I:128, B:2 * B] = xt[1::2]
        xst = np.ascontiguousarray(
            xs4.transpose(1, 0, 2)).reshape(128, NPAIR * I)
        in_maps.append({"wt": wt, "xst": xst, "diag": diag, "diagt": diagt})
    return in_maps


_CACHED_NC = None


def _install_ntff_hook():
    """Provide antenv.axon_hooks.get_axon_ntff_profile_hook when the agent
    image lacks it, by driving the injected libaxon_pjrt.so directly."""
    import types
    import ctypes
    import contextlib
    try:
        from antenv.axon_hooks import get_axon_ntff_profile_hook  # noqa: F401
        return True
    except ImportError:
        pass
    so_path = "/opt/axon/libaxon_pjrt.so"
    if not os.path.exists(so_path):
        return False
    lib = ctypes.CDLL(so_path)
    if not hasattr(lib, "axon_start_nrt_profile"):
        return False
    lib.axon_start_nrt_profile.argtypes = [
        ctypes.POINTER(ctypes.c_int64), ctypes.c_size_t]
    lib.axon_start_nrt_profile.restype = ctypes.c_int64
    lib.axon_stop_nrt_profile.argtypes = [ctypes.c_char_p]
    lib.axon_stop_nrt_profile.restype = ctypes.c_int64

    @contextlib.contextmanager
    def _hook(output_dir, device_ids):
        import jax
        jax.devices()
        if device_ids:
            ids = (ctypes.c_int64 * len(device_ids))(*device_ids)
            rc = lib.axon_start_nrt_profile(ids, len(device_ids))
        else:
            rc = lib.axon_start_nrt_profile(None, 0)
        if rc != 0:
            raise RuntimeError(f"axon_start_nrt_profile rc={rc}")
        try:
            yield
        finally:
            n = lib.axon_stop_nrt_profile(str(output_dir).encode())
            if n < 0:
                raise RuntimeError(f"axon_stop_nrt_profile rc={n}")

    import antenv
    mod = types.ModuleType("antenv.axon_hooks")
    mod.get_axon_ntff_profile_hook = lambda: _hook
    mod.set_axon_ntff_profile_hook = lambda h: None
    sys.modules["antenv.axon_hooks"] = mod
    antenv.axon_hooks = mod
    return True


def kernel(inputs, W):
    global _CACHED_NC
    inputs = np.asarray(inputs)
    W = np.asarray(W)
    if _CACHED_NC is None:
        _CACHED_NC = build_program()
    nc = _CACHED_NC
    in_maps = pack_inputs(inputs, W)
    trace = bool(int(os.environ.get("CAPS_TRACE", "0")))
    if trace:
        trace = _install_ntff_hook()
    res = bass_utils.run_bass_kernel_spmd(
        nc, in_maps, core_ids=list(range(N_CORES)), trace=trace,
    )
    kernel.last_results = res
    if trace and res.exec_time_ns is not None:
        print(f"HW exec time: {res.exec_time_ns} ns", file=sys.stderr)
        kernel.last_exec_time_ns = res.exec_time_ns
    out = res.results[0]["out"]  # [B, DK] fp32, identical on all cores
    return np.ascontiguousarray(
        out.reshape(B, D, K).transpose(0, 2, 1)
    ).astype(np.float32)


kernel.last_exec_time_ns = None
kernel.last_results = None
